# revision 14
# baseline (speedup 1.0000x reference)
"""Cross-attention kernel for TRN2, 8-core SPMD.

Reference op (B=4, T=2048, S=512, D=1024, H=16, Hd=64):
    q = (x @ Wq + bq); k,v = context @ Wkv + bkv
    out = softmax(q k^T / sqrt(Hd) + mask) @ v @ Wp + bp

Sharding: pure data-parallel over (batch, T/2): core c owns batch c//2,
query rows (c%2)*1024..+1024.  Each core recomputes K/V for its batch
(2x duplicated KV-proj work, zero collectives).  Weights replicated.

v4 schedule (per core, R=1024 query rows).  The ACT Exp stream over the
score matrix (8.4M elements, ~90us at 128 lanes x 1.2GHz) is the pacing
engine for the attention inner loop, so everything is organized to
start it early and keep it dense:
  - head-streamed pipeline: k_proj(0..6) leads (only needs ctx+wk,
    ~3MB), then per head-pair q_proj -> QK -> Exp, with v_proj and
    av(hp-2) woven in; all independent PE work rides in the PE queue
    between exp-gated QK matmuls.
  - ALL psum tiles for projections and QK share one single-bank
    [128,512] pool (6 bufs) so the Exp stream is never throttled by
    psum recycling (a 2-buf double-bank QK pool serializes exp ->
    QK(s+1) -> exp at ~2.3x the exp cost).
  - input DMAs: each DGE queue sustains only ~150GB/s, so chunks are
    spread over the 3 queues (sync/SP, scalar/ACT, gpsimd) ordered by
    compute deadline; y output DMAs rotate over all 3 queues.
  - softmax denominators ride the AV matmuls for free (ones columns);
    sums are evicted to SBUF fp16 with a cross-partition DVE copy (the
    offset write aligns each head's sums with its O rows, verified on
    HW), one batched in-place ACT Reciprocal per rc-half (a single
    Exp->Recip LUT switch per kernel; v1 paid 12 table loads and a
    24us HAM re-throttle), recip(rc0) overlapping av(7,rc1) on the PE.
  - phase D (Y = OT^T @ Wp + bp) starts as soon as the rc0 rows are
    normalized; evictions + DMAs interleave; rc1 normalize hides under
    the first D row-blocks.

Layouts as v1: all activations transposed (feature on partitions), no
on-chip transposes; KT [D,S], QT [D,R] fp16; V_aug fp16 [S, 8x192] =
[V_even|ones|V_odd] with wv/ones pre-scaled 2^-10 so unnormalized
attention outputs fit fp16; QK contracts Hd=64 with two heads in PE
row-groups 0/64 concurrently; no max-subtraction (|scores| <= ~8).

Numerics: fp16 operands, fp32 PSUM accumulation.  Max-abs error vs the
fp32 reference ~1.1e-3 of max|out|.
"""
import os
import sys
import types

import numpy as np

import concourse.tile as tile
from concourse import bacc, mybir
from concourse.bass_utils import run_bass_kernel_spmd

F32 = mybir.dt.float32
F16 = mybir.dt.float16
AF = mybir.ActivationFunctionType

B, T, S, D = 4, 2048, 512, 1024
H, HD = 16, 64
NCORE = 8
R = B * T // NCORE          # 1024 query rows per core
KC = D // 128               # 8 contraction chunks
SC = S // 128               # 4 context chunks
NP = H // 2                 # 8 head pairs
NEG = -60.0                 # mask bias (exp(-60) ~ 0)

_CACHE = {}
last_results = None         # BassKernelResults of the most recent run


def _install_ntff_hook():
    """antenv.axon_hooks is absent in this image; recreate it from the
    boot helper so BASS_TRACE=1 profiling works. Best-effort."""
    try:
        import antenv.axon_hooks  # noqa: F401
        return
    except ImportError:
        pass
    try:
        from trn_agent_boot.trn_boot import _ntff_profile_via_ctypes
        hook = _ntff_profile_via_ctypes("/opt/axon/libaxon_pjrt.so")
        mod = types.ModuleType("antenv.axon_hooks")
        mod.get_axon_ntff_profile_hook = lambda: hook
        sys.modules["antenv.axon_hooks"] = mod
    except Exception:
        pass


_install_ntff_hook()


def _act_recip(nc, out_ap, in_ap):
    """Raw ACT Reciprocal (bass blocks the helper for accuracy reasons;
    measured ~1e-5 rel err here, plenty for softmax denominators)."""
    eng = nc.scalar
    return eng.add_instruction(
        mybir.InstActivation(
            name=nc.get_next_instruction_name(),
            func=AF.Reciprocal,
            ins=[eng.lower_ap(in_ap),
                 mybir.ImmediateValue(dtype=F32, value=0.0),
                 mybir.ImmediateValue(dtype=F32, value=1.0),
                 mybir.ImmediateValue(dtype=F32, value=0.0)],
            outs=[eng.lower_ap(out_ap)],
        ))


def _build():
    nc = bacc.Bacc("TRN2", target_bir_lowering=False, debug=False,
                   num_devices=NCORE)

    # all big inputs are packed host-side to [128, KC*X] (partition-
    # major) so each loads as ONE DMA with 8-16KB contiguous lines --
    # the DGE is per-descriptor-bound, 1-2KB lines only reach half rate
    xT = nc.dram_tensor("xT", [128, KC * R], F16, kind="ExternalInput").ap()
    ctxT = nc.dram_tensor("ctxT", [128, KC * S], F16, kind="ExternalInput").ap()
    maskb = nc.dram_tensor("maskb", [128, SC], F32, kind="ExternalInput").ap()
    wq = nc.dram_tensor("wq", [128, KC * D], F16, kind="ExternalInput").ap()
    bq = nc.dram_tensor("bq", [128, KC], F32, kind="ExternalInput").ap()
    wk = nc.dram_tensor("wk", [128, KC * D], F16, kind="ExternalInput").ap()
    bk = nc.dram_tensor("bk", [128, KC], F32, kind="ExternalInput").ap()
    wv = nc.dram_tensor("wv", [128, KC * D], F16, kind="ExternalInput").ap()
    wp = nc.dram_tensor("wp", [128, KC * D], F16, kind="ExternalInput").ap()
    bp_r = nc.dram_tensor("bp_r", [128, D], F32, kind="ExternalInput").ap()
    y = nc.dram_tensor("y", [R, D], F32, kind="ExternalOutput").ap()

    with tile.TileContext(nc) as tc:
        # Pool stack bottom -> top; exp/psum pools on top so they close
        # after the attention stream, freeing room for psD / y.
        p_const = tc.tile_pool(name="const", bufs=1)
        p_kv = tc.tile_pool(name="kv", bufs=1)
        p_qt = tc.tile_pool(name="qt", bufs=1)
        p_ot = tc.tile_pool(name="ot", bufs=1)
        p_wp = tc.tile_pool(name="wpp", bufs=1)
        p_sums = tc.tile_pool(name="sums", bufs=1)
        p_ld = tc.tile_pool(name="ld", bufs=1)
        p_exp = tc.tile_pool(name="exp", bufs=44)
        p_ps = tc.tile_pool(name="ps", bufs=6, space="PSUM")
        p_psAV = tc.tile_pool(name="psAV", bufs=2, space="PSUM")
        constp = p_const.__enter__()
        kvp = p_kv.__enter__()
        qtp = p_qt.__enter__()
        otp = p_ot.__enter__()
        wpp = p_wp.__enter__()
        sumsp = p_sums.__enter__()
        ldp = p_ld.__enter__()
        expp = p_exp.__enter__()
        psp = p_ps.__enter__()
        psAV = p_psAV.__enter__()

        # ---- PE warm-up on a memset tile: covers the initial DMA
        # window and gets HAM to K=8/8 before real work ----
        warm_sb = constp.tile([128, 512], F16, tag="warm_sb")
        nc.vector.memset(warm_sb[:], 0.0)
        warm_ps = psp.tile([128, 512], F32, tag="ps")
        for w in range(20):
            nc.tensor.matmul(warm_ps[:], warm_sb[:, 0:128], warm_sb[:],
                             start=True, stop=True, skip_group_check=True)

        # ---- persistent tiles (one big tile per input; chunk views) ----
        xT_b = ldp.tile([128, KC, R], F16, tag="xTb")
        wq_b = ldp.tile([128, KC, D], F16, tag="wqb")
        wk_b = ldp.tile([128, KC, D], F16, tag="wkb")
        wv_b = ldp.tile([128, KC, D], F16, tag="wvb")
        ctx_b = ldp.tile([128, KC, S], F16, tag="ctxb")
        xT_t = [xT_b[:, k, :] for k in range(KC)]
        wq_t = [wq_b[:, k, :] for k in range(KC)]
        wk_t = [wk_b[:, k, :] for k in range(KC)]
        wv_t = [wv_b[:, k, :] for k in range(KC)]
        ctx_t = [ctx_b[:, k, :] for k in range(KC)]
        mb_t = constp.tile([128, SC], F32, tag="mb")
        bq_t = constp.tile([128, KC], F32, tag="bq")
        bk_t = constp.tile([128, KC], F32, tag="bk")
        bp_t = constp.tile([128, D], F32, tag="bp")
        wp_b = wpp.tile([128, KC, D], F16, tag="wpb")
        wp_t = [wp_b[:, k, :] for k in range(KC)]
        KT = [kvp.tile([128, S], F16, tag=f"KT{m}", name=f"KT{m}")
              for m in range(KC)]
        # V_aug: [128, pair, 192] = [V_even | ones(64) | V_odd]
        VA = [kvp.tile([128, NP, 192], F16, tag=f"VA{s}", name=f"VA{s}")
              for s in range(SC)]
        QT = [qtp.tile([128, R], F16, tag=f"QT{m}", name=f"QT{m}")
              for m in range(KC)]
        OT = [otp.tile([128, R], F16, tag=f"OT{m}", name=f"OT{m}")
              for m in range(KC)]
        # softmax denominators, head-aligned with OT rows: [rc, hp]
        sums_t = sumsp.tile([128, 2, NP, 512], F16, tag="sums")

        # ---- input DMAs: one big transfer per tensor, spread over the
        # 3 DGE queues by compute deadline: (wk,ctx) -> k_proj, then
        # (xT,wq) -> q_proj, then wv, wp, bp.  The ones blocks of V_aug
        # come from a DVE memset instead of a load.
        nc.sync.dma_start(wk_b[:], wk[:])
        nc.sync.dma_start(bp_t[:], bp_r[:])
        nc.scalar.dma_start(bq_t[:], bq[:])
        nc.scalar.dma_start(bk_t[:], bk[:])
        nc.scalar.dma_start(mb_t[:], maskb[:])
        nc.scalar.dma_start(ctx_b[:], ctxT[:])
        nc.scalar.dma_start(wq_b[:], wq[:])
        nc.gpsimd.dma_start(xT_b[:], xT[:])
        nc.gpsimd.dma_start(wv_b[:], wv[:])
        nc.gpsimd.dma_start(wp_b[:], wp[:])
        for s in range(SC):
            nc.vector.memset(VA[s][:, :, 64:128], 2.0 ** -10)

        def q_proj(m, rc):
            ps = psp.tile([128, 512], F32, tag="ps")
            for k in range(KC):
                nc.tensor.matmul(
                    ps[:], wq_t[k][:, m * 128:(m + 1) * 128],
                    xT_t[k][:, rc * 512:(rc + 1) * 512],
                    start=(k == 0), stop=(k == KC - 1))
            nc.vector.tensor_scalar_add(
                QT[m][:, rc * 512:(rc + 1) * 512], ps[:], bq_t[:, m:m + 1])

        def k_proj(m):
            ps = psp.tile([128, S], F32, tag="ps")
            for k in range(KC):
                nc.tensor.matmul(ps[:], wk_t[k][:, m * 128:(m + 1) * 128],
                                 ctx_t[k][:],
                                 start=(k == 0), stop=(k == KC - 1))
            nc.vector.tensor_scalar_add(KT[m][:], ps[:], bk_t[:, m:m + 1])

        def v_proj(n, s):
            ps = psp.tile([128, 512], F32, tag="ps")
            for k in range(KC):
                nc.tensor.matmul(ps[:], ctx_t[k][:, s * 128:(s + 1) * 128],
                                 wv_t[k][:, n * 512:(n + 1) * 512],
                                 start=(k == 0), stop=(k == KC - 1))
            # scatter 8 heads (4 pairs) into V_aug blocks
            vsrc = ps[:].rearrange("p (h c) -> p h c", c=64)
            nc.vector.tensor_copy(VA[s][:, 4 * n:4 * n + 4, 0:64],
                                  vsrc[:, 0::2, :])
            nc.vector.tensor_copy(VA[s][:, 4 * n:4 * n + 4, 128:192],
                                  vsrc[:, 1::2, :])

        def qk_slot(hp, ex, rc, s):
            """One (rc, s) score block: 2 concurrent row-group matmuls
            + 2 Exps.  Fill work is emitted between slots so the PE
            FIFO never head-of-line-blocks the ACT Exp stream."""
            pss = []
            for e in range(2):
                lo, hi = 64 * e, 64 * e + 64
                ps = psp.tile([128, 512], F32, tag="ps",
                              name=f"psqk{hp}_{s}_{rc}_{e}")
                nc.tensor.matmul(
                    ps[:],
                    KT[hp][lo:hi, s * 128:(s + 1) * 128],
                    QT[hp][lo:hi, rc * 512:(rc + 1) * 512],
                    start=True, stop=True)
                pss.append(ps)
            for e in range(2):
                nc.scalar.activation(ex[e][rc][s][:], pss[e][:],
                                     AF.Exp, bias=mb_t[:, s:s + 1])

        def attn_av(hp, ex, rc, e):
            rr = slice(rc * 512, rc * 512 + 512)
            # even head: V cols 0:128 -> O rows 0:64, sums 64:128
            # odd  head: V cols 64:192 -> sums 0:64, O rows 64:128
            voff = 64 * e
            olo, ohi = (0, 64) if e == 0 else (64, 128)
            slo, shi = (64, 128) if e == 0 else (0, 64)
            ps = psAV.tile([128, 512], F32, tag="psAV")
            for s in range(SC):
                nc.tensor.matmul(
                    ps[:], VA[s][:, hp, voff:voff + 128],
                    ex[e][rc][s][:],
                    start=(s == 0), stop=(s == SC - 1))
            nc.vector.tensor_copy(OT[hp][olo:ohi, rr],
                                  ps[olo:ohi, :])
            # cross-partition evict: head hp's sums land in the
            # opposite 64-half; write them aligned with its O
            nc.vector.tensor_copy(sums_t[olo:ohi, rc, hp, :],
                                  ps[slo:shi, :])

        # ============ head-streamed pipeline ============
        # q_proj halves (qc), v_proj chains (vc) and av pairs (ac) are
        # dribbled between the (rc, s) score slots from this ordered
        # work list, so neither the PE FIFO nor the Exp stream ever
        # stalls the other for more than ~1us.
        def qc(m, rc):
            return lambda ex: q_proj(m, rc)

        def kc(m):
            return lambda ex: k_proj(m)

        def vc(n, s):
            return lambda ex: v_proj(n, s)

        def ac(hp, rc, e):
            return lambda ex: attn_av(hp, ex[hp], rc, e)

        work = [
            qc(0, 1), kc(2), qc(1, 0), kc(3), qc(1, 1), kc(4), kc(5),
            kc(6), kc(7), qc(2, 0), vc(0, 0), qc(2, 1), vc(0, 1),
            vc(0, 2), qc(3, 0), vc(0, 3), qc(3, 1),
            ac(0, 0, 0), ac(0, 0, 1), qc(4, 0), ac(0, 1, 0), ac(0, 1, 1),
            qc(4, 1), ac(1, 0, 0), ac(1, 0, 1), qc(5, 0), ac(1, 1, 0),
            ac(1, 1, 1), qc(5, 1), ac(2, 0, 0), ac(2, 0, 1), qc(6, 0),
            ac(2, 1, 0), ac(2, 1, 1), qc(6, 1), vc(1, 0), ac(3, 0, 0),
            ac(3, 0, 1), qc(7, 0), vc(1, 1), ac(3, 1, 0), ac(3, 1, 1),
            qc(7, 1), vc(1, 2), vc(1, 3), ac(4, 0, 0), ac(4, 0, 1),
            ac(4, 1, 0), ac(4, 1, 1), ac(5, 0, 0), ac(5, 0, 1),
            ac(5, 1, 0), ac(5, 1, 1), ac(6, 0, 0), ac(6, 0, 1),
            ac(6, 1, 0), ac(6, 1, 1), ac(7, 0, 0), ac(7, 0, 1),
        ]
        k_proj(0)
        k_proj(1)
        q_proj(0, 0)
        exs = {}
        slot = 0
        popped = 0
        for hp in range(NP):
            exs[hp] = [[[expp.tile([128, 512], F16, tag="exp",
                                   name=f"ex{hp}_{e}_{rc}_{s}")
                         for s in range(SC)] for rc in range(2)]
                       for e in range(2)]
            for rc in range(2):
                for s in range(SC):
                    qk_slot(hp, exs[hp], rc, s)
                    slot += 1
                    want = (slot * len(work)) // (NP * 2 * SC)
                    while popped < want:
                        work[popped](exs)
                        popped += 1
        while popped < len(work):
            work[popped](exs)
            popped += 1
        # recip rc0 on ACT overlaps av(7, rc1) on the PE
        _act_recip(nc, sums_t[:, 0, :, :], sums_t[:, 0, :, :])
        attn_av(NP - 1, exs[NP - 1], 1, 0)
        attn_av(NP - 1, exs[NP - 1], 1, 1)
        for hp in range(NP):
            nc.vector.tensor_mul(OT[hp][:, 0:512], OT[hp][:, 0:512],
                                 sums_t[:, 0, hp, :])
        _act_recip(nc, sums_t[:, 1, :, :], sums_t[:, 1, :, :])
        for hp in range(NP):
            nc.vector.tensor_mul(OT[hp][:, 512:1024], OT[hp][:, 512:1024],
                                 sums_t[:, 1, hp, :])

        p_psAV.__exit__(None, None, None)
        p_ps.__exit__(None, None, None)
        p_exp.__exit__(None, None, None)

        # ================= output projection =================
        p_psD = tc.tile_pool(name="psD", bufs=5, space="PSUM")
        psD = p_psD.__enter__()
        p_y = tc.tile_pool(name="y", bufs=4)
        yp = p_y.__enter__()
        dma_engs = [nc.sync, nc.gpsimd, nc.scalar]
        for rp in range(KC):
            for n in range(2):
                ps = psD.tile([128, 512], F32, tag="psD")
                for k in range(KC):
                    nc.tensor.matmul(
                        ps[:], OT[k][:, rp * 128:(rp + 1) * 128],
                        wp_t[k][:, n * 512:(n + 1) * 512],
                        start=(k == 0), stop=(k == KC - 1))
                yt = yp.tile([128, 512], F32, tag="y")
                nc.vector.tensor_add(yt[:], ps[:], bp_t[:, n * 512:(n + 1) * 512])
                i = rp * 2 + n
                dma_engs[i % 3].dma_start(
                    y[rp * 128:rp * 128 + 64, n * 512:(n + 1) * 512],
                    yt[0:64, :])
                dma_engs[(i + 1) % 3].dma_start(
                    y[rp * 128 + 64:(rp + 1) * 128, n * 512:(n + 1) * 512],
                    yt[64:128, :])
        p_y.__exit__(None, None, None)
        p_psD.__exit__(None, None, None)
        p_ld.__exit__(None, None, None)
        p_sums.__exit__(None, None, None)
        p_wp.__exit__(None, None, None)
        p_ot.__exit__(None, None, None)
        p_qt.__exit__(None, None, None)
        p_kv.__exit__(None, None, None)
        p_const.__exit__(None, None, None)

    nc.compile()
    return nc


def _get_nc():
    if "nc" not in _CACHE:
        _CACHE["nc"] = _build()
    return _CACHE["nc"]


def kernel(x, context, context_mask, Wq, bq, Wkv, bkv, Wp, bp):
    global last_results
    x = np.asarray(x, dtype=np.float32)
    context = np.asarray(context, dtype=np.float32)
    context_mask = np.asarray(context_mask)
    Wq = np.asarray(Wq, dtype=np.float32)
    bq = np.asarray(bq, dtype=np.float32)
    Wkv = np.asarray(Wkv, dtype=np.float32)
    bkv = np.asarray(bkv, dtype=np.float32)
    Wp = np.asarray(Wp, dtype=np.float32)
    bp = np.asarray(bp, dtype=np.float32)

    sc = 1.0 / np.sqrt(HD)

    def pack(a):
        # [D, X] row-chunked -> [128, KC*X] partition-major so the
        # whole tensor loads as one DMA with KC*X*2-byte lines
        Dl, X = a.shape
        return np.ascontiguousarray(
            a.reshape(Dl // 128, 128, X).transpose(1, 0, 2)
            .reshape(128, (Dl // 128) * X))

    # kv reshape in the reference is [S, 2, H, Hd]: k cols = Wkv[:, :D]
    wq_h = pack((Wq * sc).astype(np.float16))
    bq_h = np.ascontiguousarray((bq * sc).reshape(KC, 128).T)
    wk_h = pack(Wkv[:, :D].astype(np.float16))
    bk_h = np.ascontiguousarray(bkv[:D].reshape(KC, 128).T)
    wv_h = pack((Wkv[:, D:] * 2.0**-10).astype(np.float16))
    bv = bkv[D:]
    wp_h = pack(Wp.astype(np.float16))
    bp_eff = bp + bv @ Wp          # softmax rows sum to 1
    bp_r = np.ascontiguousarray(
        np.broadcast_to(bp_eff.astype(np.float32), (128, D)))

    in_maps = []
    for c in range(NCORE):
        b = c // 2
        r0 = (c % 2) * R
        in_maps.append({
            "xT": pack(x[b, r0:r0 + R, :].T.astype(np.float16)),
            "ctxT": pack(context[b].T.astype(np.float16)),
            "maskb": np.ascontiguousarray(
                np.where(context_mask[b], 0.0, NEG).astype(np.float32)
                .reshape(SC, 128).T),
            "wq": wq_h, "bq": bq_h,
            "wk": wk_h, "bk": bk_h,
            "wv": wv_h,
            "wp": wp_h, "bp_r": bp_r,
        })

    nc = _get_nc()
    res = run_bass_kernel_spmd(nc, in_maps, list(range(NCORE)),
                               trace=bool(os.environ.get("BASS_TRACE")))
    last_results = res

    out = np.empty((B, T, D), dtype=np.float32)
    for c in range(NCORE):
        b = c // 2
        r0 = (c % 2) * R
        out[b, r0:r0 + R, :] = res.results[c]["y"]
    return out


# revision 15
# speedup vs baseline: 1.0471x; 1.0471x over previous
"""Cross-attention kernel for TRN2, 8-core SPMD.

Reference op (B=4, T=2048, S=512, D=1024, H=16, Hd=64):
    q = (x @ Wq + bq); k,v = context @ Wkv + bkv
    out = softmax(q k^T / sqrt(Hd) + mask) @ v @ Wp + bp

Sharding: pure data-parallel over (batch, T/2): core c owns batch c//2,
query rows (c%2)*1024..+1024.  Each core recomputes K/V for its batch
(2x duplicated KV-proj work, zero collectives).  Weights replicated.

v4 schedule (per core, R=1024 query rows).  The ACT Exp stream over the
score matrix (8.4M elements, ~90us at 128 lanes x 1.2GHz) is the pacing
engine for the attention inner loop, so everything is organized to
start it early and keep it dense:
  - head-streamed pipeline: k_proj(0..6) leads (only needs ctx+wk,
    ~3MB), then per head-pair q_proj -> QK -> Exp, with v_proj and
    av(hp-2) woven in; all independent PE work rides in the PE queue
    between exp-gated QK matmuls.
  - ALL psum tiles for projections and QK share one single-bank
    [128,512] pool (6 bufs) so the Exp stream is never throttled by
    psum recycling (a 2-buf double-bank QK pool serializes exp ->
    QK(s+1) -> exp at ~2.3x the exp cost).
  - input DMAs: each DGE queue sustains only ~150GB/s, so chunks are
    spread over the 3 queues (sync/SP, scalar/ACT, gpsimd) ordered by
    compute deadline; y output DMAs rotate over all 3 queues.
  - softmax denominators ride the AV matmuls for free (ones columns);
    sums are evicted to SBUF fp16 with a cross-partition DVE copy (the
    offset write aligns each head's sums with its O rows, verified on
    HW), one batched in-place ACT Reciprocal per rc-half (a single
    Exp->Recip LUT switch per kernel; v1 paid 12 table loads and a
    24us HAM re-throttle), recip(rc0) overlapping av(7,rc1) on the PE.
  - phase D (Y = OT^T @ Wp + bp) starts as soon as the rc0 rows are
    normalized; evictions + DMAs interleave; rc1 normalize hides under
    the first D row-blocks.

Layouts as v1: all activations transposed (feature on partitions), no
on-chip transposes; KT [D,S], QT [D,R] fp16; V_aug fp16 [S, 8x192] =
[V_even|ones|V_odd] with wv/ones pre-scaled 2^-10 so unnormalized
attention outputs fit fp16; QK contracts Hd=64 with two heads in PE
row-groups 0/64 concurrently; no max-subtraction (|scores| <= ~8).

Numerics: fp16 operands, fp32 PSUM accumulation.  Max-abs error vs the
fp32 reference ~1.1e-3 of max|out|.
"""
import os
import sys
import types

import numpy as np

import concourse.tile as tile
from concourse import bacc, mybir
from concourse.bass_utils import run_bass_kernel_spmd

F32 = mybir.dt.float32
F16 = mybir.dt.float16
AF = mybir.ActivationFunctionType

B, T, S, D = 4, 2048, 512, 1024
H, HD = 16, 64
NCORE = 8
R = B * T // NCORE          # 1024 query rows per core
KC = D // 128               # 8 contraction chunks
SC = S // 128               # 4 context chunks
NP = H // 2                 # 8 head pairs
NEG = -60.0                 # mask bias (exp(-60) ~ 0)

_CACHE = {}
last_results = None         # BassKernelResults of the most recent run


def _install_ntff_hook():
    """antenv.axon_hooks is absent in this image; recreate it from the
    boot helper so BASS_TRACE=1 profiling works. Best-effort."""
    try:
        import antenv.axon_hooks  # noqa: F401
        return
    except ImportError:
        pass
    try:
        from trn_agent_boot.trn_boot import _ntff_profile_via_ctypes
        hook = _ntff_profile_via_ctypes("/opt/axon/libaxon_pjrt.so")
        mod = types.ModuleType("antenv.axon_hooks")
        mod.get_axon_ntff_profile_hook = lambda: hook
        sys.modules["antenv.axon_hooks"] = mod
    except Exception:
        pass


_install_ntff_hook()


def _act_recip(nc, out_ap, in_ap):
    """Raw ACT Reciprocal (bass blocks the helper for accuracy reasons;
    measured ~1e-5 rel err here, plenty for softmax denominators)."""
    eng = nc.scalar
    return eng.add_instruction(
        mybir.InstActivation(
            name=nc.get_next_instruction_name(),
            func=AF.Reciprocal,
            ins=[eng.lower_ap(in_ap),
                 mybir.ImmediateValue(dtype=F32, value=0.0),
                 mybir.ImmediateValue(dtype=F32, value=1.0),
                 mybir.ImmediateValue(dtype=F32, value=0.0)],
            outs=[eng.lower_ap(out_ap)],
        ))


def _build():
    nc = bacc.Bacc("TRN2", target_bir_lowering=False, debug=False,
                   num_devices=NCORE)

    # all big inputs are packed host-side to [128, KC*X] (partition-
    # major) so each loads as ONE DMA with 8-16KB contiguous lines --
    # the DGE is per-descriptor-bound, 1-2KB lines only reach half rate
    xT = nc.dram_tensor("xT", [128, KC * R], F16, kind="ExternalInput").ap()
    ctxT = nc.dram_tensor("ctxT", [128, KC * S], F16, kind="ExternalInput").ap()
    maskb = nc.dram_tensor("maskb", [128, SC], F32, kind="ExternalInput").ap()
    # wq/wk are column-packed [128, m, k, 128]: k_proj(m)/q_proj(m)
    # depend only on the 256KB m-th column block, not the whole matrix
    wq = nc.dram_tensor("wq", [128, KC * D], F16, kind="ExternalInput").ap()
    bq = nc.dram_tensor("bq", [128, KC], F32, kind="ExternalInput").ap()
    wk = nc.dram_tensor("wk", [128, KC * D], F16, kind="ExternalInput").ap()
    bk = nc.dram_tensor("bk", [128, KC], F32, kind="ExternalInput").ap()
    wv = nc.dram_tensor("wv", [128, KC * D], F16, kind="ExternalInput").ap()
    wp = nc.dram_tensor("wp", [128, KC * D], F16, kind="ExternalInput").ap()
    bp_r = nc.dram_tensor("bp_r", [128, D], F32, kind="ExternalInput").ap()
    y = nc.dram_tensor("y", [R, D], F32, kind="ExternalOutput").ap()

    with tile.TileContext(nc) as tc:
        # Pool stack bottom -> top; exp/psum pools on top so they close
        # after the attention stream, freeing room for psD / y.
        p_const = tc.tile_pool(name="const", bufs=1)
        p_kv = tc.tile_pool(name="kv", bufs=1)
        p_qt = tc.tile_pool(name="qt", bufs=1)
        p_ot = tc.tile_pool(name="ot", bufs=1)
        p_wp = tc.tile_pool(name="wpp", bufs=1)
        p_sums = tc.tile_pool(name="sums", bufs=1)
        p_ld = tc.tile_pool(name="ld", bufs=1)
        p_exp = tc.tile_pool(name="exp", bufs=44)
        p_ps = tc.tile_pool(name="ps", bufs=6, space="PSUM")
        p_psAV = tc.tile_pool(name="psAV", bufs=2, space="PSUM")
        constp = p_const.__enter__()
        kvp = p_kv.__enter__()
        qtp = p_qt.__enter__()
        otp = p_ot.__enter__()
        wpp = p_wp.__enter__()
        sumsp = p_sums.__enter__()
        ldp = p_ld.__enter__()
        expp = p_exp.__enter__()
        psp = p_ps.__enter__()
        psAV = p_psAV.__enter__()

        # ---- PE warm-up on a memset tile: covers the initial DMA
        # window and gets HAM to K=8/8 before real work ----
        warm_sb = constp.tile([128, 512], F16, tag="warm_sb")
        nc.vector.memset(warm_sb[:], 0.0)
        warm_ps = psp.tile([128, 512], F32, tag="ps")
        for w in range(16):
            nc.tensor.matmul(warm_ps[:], warm_sb[:, 0:128], warm_sb[:],
                             start=True, stop=True, skip_group_check=True)

        # ---- persistent tiles (big tiles; chunk views) ----
        xT_b = ldp.tile([128, KC, R], F16, tag="xTb")
        wq_b = ldp.tile([128, KC, KC, 128], F16, tag="wqb")
        wk_b = ldp.tile([128, KC, KC, 128], F16, tag="wkb")
        wv_b = ldp.tile([128, 2, KC, 512], F16, tag="wvb")
        ctx_b = ldp.tile([128, KC, S], F16, tag="ctxb")
        xT_t = [xT_b[:, k, :] for k in range(KC)]
        ctx_t = [ctx_b[:, k, :] for k in range(KC)]
        mb_t = constp.tile([128, SC], F32, tag="mb")
        bq_t = constp.tile([128, KC], F32, tag="bq")
        bk_t = constp.tile([128, KC], F32, tag="bk")
        bp_t = constp.tile([128, D], F32, tag="bp")
        wp_b = wpp.tile([128, KC, D], F16, tag="wpb")
        wp_t = [wp_b[:, k, :] for k in range(KC)]
        KT = [kvp.tile([128, S], F16, tag=f"KT{m}", name=f"KT{m}")
              for m in range(KC)]
        # V_aug: [128, pair, 192] = [V_even | ones(64) | V_odd]
        VA = [kvp.tile([128, NP, 192], F16, tag=f"VA{s}", name=f"VA{s}")
              for s in range(SC)]
        QT = [qtp.tile([128, R], F16, tag=f"QT{m}", name=f"QT{m}")
              for m in range(KC)]
        OT = [otp.tile([128, R], F16, tag=f"OT{m}", name=f"OT{m}")
              for m in range(KC)]
        # softmax denominators, head-aligned with OT rows: [rc, hp]
        sums_t = sumsp.tile([128, 2, NP, 512], F16, tag="sums")

        # ---- input DMAs, spread over the 3 DGE queues (~140GB/s each)
        # by compute deadline: ctx halves + wk column blocks feed the
        # k_proj stream from ~12us; xT halves + wq_m0 gate q_proj(0);
        # wv/wp/bp arrive under the attention stream.  V_aug's ones
        # blocks come from a DVE memset instead of a load.
        CB = KC * 128
        nc.sync.dma_start(ctx_b[:, 0:4, :], ctxT[:, 0:4 * S])
        for j in range(KC):
            nc.sync.dma_start(wk_b[:, j, :, :], wk[:, j * CB:(j + 1) * CB])
        nc.sync.dma_start(wv_b[:, 0, :, :], wv[:, 0:KC * 512])
        nc.scalar.dma_start(bq_t[:], bq[:])
        nc.scalar.dma_start(bk_t[:], bk[:])
        nc.scalar.dma_start(mb_t[:], maskb[:])
        nc.scalar.dma_start(ctx_b[:, 4:8, :], ctxT[:, 4 * S:])
        nc.scalar.dma_start(xT_b[:, 4:8, :], xT[:, 4 * R:])
        for j in range(4):
            nc.scalar.dma_start(wq_b[:, j, :, :], wq[:, j * CB:(j + 1) * CB])
        nc.scalar.dma_start(bp_t[:], bp_r[:])
        nc.gpsimd.dma_start(xT_b[:, 0:4, :], xT[:, 0:4 * R])
        for j in range(4, KC):
            nc.gpsimd.dma_start(wq_b[:, j, :, :], wq[:, j * CB:(j + 1) * CB])
        nc.gpsimd.dma_start(wv_b[:, 1, :, :], wv[:, KC * 512:])
        nc.gpsimd.dma_start(wp_b[:], wp[:])
        for s in range(SC):
            nc.vector.memset(VA[s][:, :, 64:128], 2.0 ** -10)

        def q_proj(m, rc):
            ps = psp.tile([128, 512], F32, tag="ps")
            for k in range(KC):
                nc.tensor.matmul(
                    ps[:], wq_b[:, m, k, :],
                    xT_t[k][:, rc * 512:(rc + 1) * 512],
                    start=(k == 0), stop=(k == KC - 1))
            nc.vector.tensor_scalar_add(
                QT[m][:, rc * 512:(rc + 1) * 512], ps[:], bq_t[:, m:m + 1])

        def k_proj(m):
            ps = psp.tile([128, S], F32, tag="ps")
            for k in range(KC):
                nc.tensor.matmul(ps[:], wk_b[:, m, k, :],
                                 ctx_t[k][:],
                                 start=(k == 0), stop=(k == KC - 1))
            nc.vector.tensor_scalar_add(KT[m][:], ps[:], bk_t[:, m:m + 1])

        def v_proj(n, s):
            ps = psp.tile([128, 512], F32, tag="ps")
            for k in range(KC):
                nc.tensor.matmul(ps[:], ctx_t[k][:, s * 128:(s + 1) * 128],
                                 wv_b[:, n, k, :],
                                 start=(k == 0), stop=(k == KC - 1))
            # scatter 8 heads (4 pairs) into V_aug blocks
            vsrc = ps[:].rearrange("p (h c) -> p h c", c=64)
            nc.vector.tensor_copy(VA[s][:, 4 * n:4 * n + 4, 0:64],
                                  vsrc[:, 0::2, :])
            nc.vector.tensor_copy(VA[s][:, 4 * n:4 * n + 4, 128:192],
                                  vsrc[:, 1::2, :])

        def qk_slot(hp, ex, rc, s):
            """One (rc, s) score block: 2 concurrent row-group matmuls
            + 2 Exps.  Fill work is emitted between slots so the PE
            FIFO never head-of-line-blocks the ACT Exp stream."""
            pss = []
            for e in range(2):
                lo, hi = 64 * e, 64 * e + 64
                ps = psp.tile([128, 512], F32, tag="ps",
                              name=f"psqk{hp}_{s}_{rc}_{e}")
                nc.tensor.matmul(
                    ps[:],
                    KT[hp][lo:hi, s * 128:(s + 1) * 128],
                    QT[hp][lo:hi, rc * 512:(rc + 1) * 512],
                    start=True, stop=True)
                pss.append(ps)
            for e in range(2):
                nc.scalar.activation(ex[e][rc][s][:], pss[e][:],
                                     AF.Exp, bias=mb_t[:, s:s + 1])

        def attn_av(hp, ex, rc, e):
            rr = slice(rc * 512, rc * 512 + 512)
            # even head: V cols 0:128 -> O rows 0:64, sums 64:128
            # odd  head: V cols 64:192 -> sums 0:64, O rows 64:128
            voff = 64 * e
            olo, ohi = (0, 64) if e == 0 else (64, 128)
            slo, shi = (64, 128) if e == 0 else (0, 64)
            ps = psAV.tile([128, 512], F32, tag="psAV")
            for s in range(SC):
                nc.tensor.matmul(
                    ps[:], VA[s][:, hp, voff:voff + 128],
                    ex[e][rc][s][:],
                    start=(s == 0), stop=(s == SC - 1))
            nc.vector.tensor_copy(OT[hp][olo:ohi, rr],
                                  ps[olo:ohi, :])
            # cross-partition evict: head hp's sums land in the
            # opposite 64-half; write them aligned with its O
            nc.vector.tensor_copy(sums_t[olo:ohi, rc, hp, :],
                                  ps[slo:shi, :])

        # ============ head-streamed pipeline ============
        # q_proj halves (qc), v_proj chains (vc) and av pairs (ac) are
        # dribbled between the (rc, s) score slots from this ordered
        # work list, so neither the PE FIFO nor the Exp stream ever
        # stalls the other for more than ~1us.
        def qc(m, rc):
            return lambda ex: q_proj(m, rc)

        def kc(m):
            return lambda ex: k_proj(m)

        def vc(n, s):
            return lambda ex: v_proj(n, s)

        def ac(hp, rc, e):
            return lambda ex: attn_av(hp, ex[hp], rc, e)

        work = [
            qc(0, 1), kc(4), qc(1, 0), kc(5), qc(1, 1), kc(6),
            kc(7), qc(2, 0), vc(0, 0), qc(2, 1), vc(0, 1),
            vc(0, 2), qc(3, 0), vc(0, 3), qc(3, 1),
            ac(0, 0, 0), ac(0, 0, 1), qc(4, 0), ac(0, 1, 0), ac(0, 1, 1),
            qc(4, 1), ac(1, 0, 0), ac(1, 0, 1), qc(5, 0), ac(1, 1, 0),
            ac(1, 1, 1), qc(5, 1), ac(2, 0, 0), ac(2, 0, 1), qc(6, 0),
            ac(2, 1, 0), ac(2, 1, 1), qc(6, 1), vc(1, 0), ac(3, 0, 0),
            ac(3, 0, 1), qc(7, 0), vc(1, 1), ac(3, 1, 0), ac(3, 1, 1),
            qc(7, 1), vc(1, 2), vc(1, 3), ac(4, 0, 0), ac(4, 0, 1),
            ac(4, 1, 0), ac(4, 1, 1), ac(5, 0, 0), ac(5, 0, 1),
            ac(5, 1, 0), ac(5, 1, 1), ac(6, 0, 0), ac(6, 0, 1),
            ac(6, 1, 0), ac(6, 1, 1), ac(7, 0, 0), ac(7, 0, 1),
        ]
        for m in range(4):
            k_proj(m)
        q_proj(0, 0)
        exs = {}
        slot = 0
        popped = 0
        for hp in range(NP):
            exs[hp] = [[[expp.tile([128, 512], F16, tag="exp",
                                   name=f"ex{hp}_{e}_{rc}_{s}")
                         for s in range(SC)] for rc in range(2)]
                       for e in range(2)]
            for rc in range(2):
                for s in range(SC):
                    qk_slot(hp, exs[hp], rc, s)
                    slot += 1
                    want = (slot * len(work)) // (NP * 2 * SC)
                    while popped < want:
                        work[popped](exs)
                        popped += 1
        while popped < len(work):
            work[popped](exs)
            popped += 1
        # recip rc0 on ACT overlaps av(7, rc1) on the PE
        _act_recip(nc, sums_t[:, 0, :, :], sums_t[:, 0, :, :])
        attn_av(NP - 1, exs[NP - 1], 1, 0)
        attn_av(NP - 1, exs[NP - 1], 1, 1)
        for hp in range(NP):
            nc.vector.tensor_mul(OT[hp][:, 0:512], OT[hp][:, 0:512],
                                 sums_t[:, 0, hp, :])
        _act_recip(nc, sums_t[:, 1, :, :], sums_t[:, 1, :, :])
        for hp in range(NP):
            nc.vector.tensor_mul(OT[hp][:, 512:1024], OT[hp][:, 512:1024],
                                 sums_t[:, 1, hp, :])

        p_psAV.__exit__(None, None, None)
        p_ps.__exit__(None, None, None)
        p_exp.__exit__(None, None, None)

        # ================= output projection =================
        p_psD = tc.tile_pool(name="psD", bufs=5, space="PSUM")
        psD = p_psD.__enter__()
        p_y = tc.tile_pool(name="y", bufs=4)
        yp = p_y.__enter__()
        dma_engs = [nc.sync, nc.gpsimd, nc.scalar]
        for rp in range(KC):
            for n in range(2):
                ps = psD.tile([128, 512], F32, tag="psD")
                for k in range(KC):
                    nc.tensor.matmul(
                        ps[:], OT[k][:, rp * 128:(rp + 1) * 128],
                        wp_t[k][:, n * 512:(n + 1) * 512],
                        start=(k == 0), stop=(k == KC - 1))
                yt = yp.tile([128, 512], F32, tag="y")
                nc.vector.tensor_add(yt[:], ps[:], bp_t[:, n * 512:(n + 1) * 512])
                i = rp * 2 + n
                dma_engs[i % 3].dma_start(
                    y[rp * 128:rp * 128 + 64, n * 512:(n + 1) * 512],
                    yt[0:64, :])
                dma_engs[(i + 1) % 3].dma_start(
                    y[rp * 128 + 64:(rp + 1) * 128, n * 512:(n + 1) * 512],
                    yt[64:128, :])
        p_y.__exit__(None, None, None)
        p_psD.__exit__(None, None, None)
        p_ld.__exit__(None, None, None)
        p_sums.__exit__(None, None, None)
        p_wp.__exit__(None, None, None)
        p_ot.__exit__(None, None, None)
        p_qt.__exit__(None, None, None)
        p_kv.__exit__(None, None, None)
        p_const.__exit__(None, None, None)

    nc.compile()
    return nc


def _get_nc():
    if "nc" not in _CACHE:
        _CACHE["nc"] = _build()
    return _CACHE["nc"]


def kernel(x, context, context_mask, Wq, bq, Wkv, bkv, Wp, bp):
    global last_results
    x = np.asarray(x, dtype=np.float32)
    context = np.asarray(context, dtype=np.float32)
    context_mask = np.asarray(context_mask)
    Wq = np.asarray(Wq, dtype=np.float32)
    bq = np.asarray(bq, dtype=np.float32)
    Wkv = np.asarray(Wkv, dtype=np.float32)
    bkv = np.asarray(bkv, dtype=np.float32)
    Wp = np.asarray(Wp, dtype=np.float32)
    bp = np.asarray(bp, dtype=np.float32)

    sc = 1.0 / np.sqrt(HD)

    def pack(a):
        # [D, X] row-chunked -> [128, KC, X] partition-major: big
        # contiguous DMA lines
        Dl, X = a.shape
        return np.ascontiguousarray(
            a.reshape(Dl // 128, 128, X).transpose(1, 0, 2)
            .reshape(128, (Dl // 128) * X))

    def pack_cols(a, w):
        # [D, D2] -> [128, j, k, w] so column block j (all row chunks
        # k) is one contiguous 2*KC*w*128-byte DMA
        Dl, D2 = a.shape
        return np.ascontiguousarray(
            a.reshape(Dl // 128, 128, D2 // w, w).transpose(1, 2, 0, 3)
            .reshape(128, -1))

    # kv reshape in the reference is [S, 2, H, Hd]: k cols = Wkv[:, :D]
    wq_h = pack_cols((Wq * sc).astype(np.float16), 128)
    bq_h = np.ascontiguousarray((bq * sc).reshape(KC, 128).T)
    wk_h = pack_cols(Wkv[:, :D].astype(np.float16), 128)
    bk_h = np.ascontiguousarray(bkv[:D].reshape(KC, 128).T)
    wv_h = pack_cols((Wkv[:, D:] * 2.0**-10).astype(np.float16), 512)
    bv = bkv[D:]
    wp_h = pack(Wp.astype(np.float16))
    bp_eff = bp + bv @ Wp          # softmax rows sum to 1
    bp_r = np.ascontiguousarray(
        np.broadcast_to(bp_eff.astype(np.float32), (128, D)))

    in_maps = []
    for c in range(NCORE):
        b = c // 2
        r0 = (c % 2) * R
        in_maps.append({
            "xT": pack(x[b, r0:r0 + R, :].T.astype(np.float16)),
            "ctxT": pack(context[b].T.astype(np.float16)),
            "maskb": np.ascontiguousarray(
                np.where(context_mask[b], 0.0, NEG).astype(np.float32)
                .reshape(SC, 128).T),
            "wq": wq_h, "bq": bq_h,
            "wk": wk_h, "bk": bk_h,
            "wv": wv_h,
            "wp": wp_h, "bp_r": bp_r,
        })

    nc = _get_nc()
    res = run_bass_kernel_spmd(nc, in_maps, list(range(NCORE)),
                               trace=bool(os.environ.get("BASS_TRACE")))
    last_results = res

    out = np.empty((B, T, D), dtype=np.float32)
    for c in range(NCORE):
        b = c // 2
        r0 = (c % 2) * R
        out[b, r0:r0 + R, :] = res.results[c]["y"]
    return out


# revision 16
# speedup vs baseline: 1.1097x; 1.0598x over previous
"""Cross-attention kernel for TRN2, 8-core SPMD.

Reference op (B=4, T=2048, S=512, D=1024, H=16, Hd=64):
    q = (x @ Wq + bq); k,v = context @ Wkv + bkv
    out = softmax(q k^T / sqrt(Hd) + mask) @ v @ Wp + bp

Sharding: pure data-parallel over (batch, T/2): core c owns batch c//2,
query rows (c%2)*1024..+1024.  Each core recomputes K/V for its batch
(2x duplicated KV-proj work, zero collectives).  Weights replicated.

v4 schedule (per core, R=1024 query rows).  The ACT Exp stream over the
score matrix (8.4M elements, ~90us at 128 lanes x 1.2GHz) is the pacing
engine for the attention inner loop, so everything is organized to
start it early and keep it dense:
  - head-streamed pipeline: k_proj(0..6) leads (only needs ctx+wk,
    ~3MB), then per head-pair q_proj -> QK -> Exp, with v_proj and
    av(hp-2) woven in; all independent PE work rides in the PE queue
    between exp-gated QK matmuls.
  - ALL psum tiles for projections and QK share one single-bank
    [128,512] pool (6 bufs) so the Exp stream is never throttled by
    psum recycling (a 2-buf double-bank QK pool serializes exp ->
    QK(s+1) -> exp at ~2.3x the exp cost).
  - input DMAs: each DGE queue sustains only ~150GB/s, so chunks are
    spread over the 3 queues (sync/SP, scalar/ACT, gpsimd) ordered by
    compute deadline; y output DMAs rotate over all 3 queues.
  - softmax denominators ride the AV matmuls for free (ones columns);
    sums are evicted to SBUF fp16 with a cross-partition DVE copy (the
    offset write aligns each head's sums with its O rows, verified on
    HW), one batched in-place ACT Reciprocal per rc-half (a single
    Exp->Recip LUT switch per kernel; v1 paid 12 table loads and a
    24us HAM re-throttle), recip(rc0) overlapping av(7,rc1) on the PE.
  - phase D (Y = OT^T @ Wp + bp) starts as soon as the rc0 rows are
    normalized; evictions + DMAs interleave; rc1 normalize hides under
    the first D row-blocks.

Layouts as v1: all activations transposed (feature on partitions), no
on-chip transposes; KT [D,S], QT [D,R] fp16; V_aug fp16 [S, 8x192] =
[V_even|ones|V_odd] with wv/ones pre-scaled 2^-10 so unnormalized
attention outputs fit fp16; QK contracts Hd=64 with two heads in PE
row-groups 0/64 concurrently; no max-subtraction (|scores| <= ~8).

Numerics: fp16 operands, fp32 PSUM accumulation.  Max-abs error vs the
fp32 reference ~1.1e-3 of max|out|.
"""
import os
import sys
import types

import numpy as np

import concourse.tile as tile
from concourse import bacc, mybir
from concourse.bass_utils import run_bass_kernel_spmd

F32 = mybir.dt.float32
F16 = mybir.dt.float16
AF = mybir.ActivationFunctionType

B, T, S, D = 4, 2048, 512, 1024
H, HD = 16, 64
NCORE = 8
R = B * T // NCORE          # 1024 query rows per core
KC = D // 128               # 8 contraction chunks
SC = S // 128               # 4 context chunks
NP = H // 2                 # 8 head pairs
NEG = -60.0                 # mask bias (exp(-60) ~ 0)

_CACHE = {}
last_results = None         # BassKernelResults of the most recent run


def _install_ntff_hook():
    """antenv.axon_hooks is absent in this image; recreate it from the
    boot helper so BASS_TRACE=1 profiling works. Best-effort."""
    try:
        import antenv.axon_hooks  # noqa: F401
        return
    except ImportError:
        pass
    try:
        from trn_agent_boot.trn_boot import _ntff_profile_via_ctypes
        hook = _ntff_profile_via_ctypes("/opt/axon/libaxon_pjrt.so")
        mod = types.ModuleType("antenv.axon_hooks")
        mod.get_axon_ntff_profile_hook = lambda: hook
        sys.modules["antenv.axon_hooks"] = mod
    except Exception:
        pass


_install_ntff_hook()


def _act_recip(nc, out_ap, in_ap):
    """Raw ACT Reciprocal (bass blocks the helper for accuracy reasons;
    measured ~1e-5 rel err here, plenty for softmax denominators)."""
    eng = nc.scalar
    return eng.add_instruction(
        mybir.InstActivation(
            name=nc.get_next_instruction_name(),
            func=AF.Reciprocal,
            ins=[eng.lower_ap(in_ap),
                 mybir.ImmediateValue(dtype=F32, value=0.0),
                 mybir.ImmediateValue(dtype=F32, value=1.0),
                 mybir.ImmediateValue(dtype=F32, value=0.0)],
            outs=[eng.lower_ap(out_ap)],
        ))


def _build():
    nc = bacc.Bacc("TRN2", target_bir_lowering=False, debug=False,
                   num_devices=NCORE)

    # all big inputs are packed host-side to [128, KC*X] (partition-
    # major) so each loads as ONE DMA with 8-16KB contiguous lines --
    # the DGE is per-descriptor-bound, 1-2KB lines only reach half rate
    xT = nc.dram_tensor("xT", [128, KC * R], F16, kind="ExternalInput").ap()
    ctxT = nc.dram_tensor("ctxT", [128, KC * S], F16, kind="ExternalInput").ap()
    maskb = nc.dram_tensor("maskb", [128, SC], F32, kind="ExternalInput").ap()
    # wq/wk are column-packed [128, m, k, 128]: k_proj(m)/q_proj(m)
    # depend only on the 256KB m-th column block, not the whole matrix
    wq = nc.dram_tensor("wq", [128, KC * D], F16, kind="ExternalInput").ap()
    bq = nc.dram_tensor("bq", [128, KC], F32, kind="ExternalInput").ap()
    wk = nc.dram_tensor("wk", [128, KC * D], F16, kind="ExternalInput").ap()
    bk = nc.dram_tensor("bk", [128, KC], F32, kind="ExternalInput").ap()
    wv = nc.dram_tensor("wv", [128, KC * D], F16, kind="ExternalInput").ap()
    wp = nc.dram_tensor("wp", [128, KC * D], F16, kind="ExternalInput").ap()
    bp_r = nc.dram_tensor("bp_r", [128, D], F32, kind="ExternalInput").ap()
    y = nc.dram_tensor("y", [R, D], F32, kind="ExternalOutput").ap()

    with tile.TileContext(nc) as tc:
        # Pool stack bottom -> top; exp/psum pools on top so they close
        # after the attention stream, freeing room for psD / y.
        p_const = tc.tile_pool(name="const", bufs=1)
        p_kv = tc.tile_pool(name="kv", bufs=1)
        p_qt = tc.tile_pool(name="qt", bufs=1)
        p_ot = tc.tile_pool(name="ot", bufs=1)
        p_wp = tc.tile_pool(name="wpp", bufs=1)
        p_sums = tc.tile_pool(name="sums", bufs=1)
        p_ld = tc.tile_pool(name="ld", bufs=1)
        p_exp = tc.tile_pool(name="exp", bufs=44)
        p_ps = tc.tile_pool(name="ps", bufs=6, space="PSUM")
        p_psAV = tc.tile_pool(name="psAV", bufs=2, space="PSUM")
        constp = p_const.__enter__()
        kvp = p_kv.__enter__()
        qtp = p_qt.__enter__()
        otp = p_ot.__enter__()
        wpp = p_wp.__enter__()
        sumsp = p_sums.__enter__()
        ldp = p_ld.__enter__()
        expp = p_exp.__enter__()
        psp = p_ps.__enter__()
        psAV = p_psAV.__enter__()

        # ---- PE warm-up on a memset tile: covers the initial DMA
        # window and gets HAM to K=8/8 before real work ----
        warm_sb = constp.tile([128, 512], F16, tag="warm_sb")
        nc.vector.memset(warm_sb[:], 0.0)
        warm_ps = psp.tile([128, 512], F32, tag="ps")
        for w in range(16):
            nc.tensor.matmul(warm_ps[:], warm_sb[:, 0:128], warm_sb[:],
                             start=True, stop=True, skip_group_check=True)

        # ---- persistent tiles (big tiles; chunk views) ----
        xT_b = ldp.tile([128, KC, R], F16, tag="xTb")
        wq_b = ldp.tile([128, KC, KC, 128], F16, tag="wqb")
        wk_b = ldp.tile([128, KC, KC, 128], F16, tag="wkb")
        wv_b = ldp.tile([128, 2, KC, 512], F16, tag="wvb")
        ctx_b = ldp.tile([128, KC, S], F16, tag="ctxb")
        xT_t = [xT_b[:, k, :] for k in range(KC)]
        ctx_t = [ctx_b[:, k, :] for k in range(KC)]
        mb_t = constp.tile([128, SC], F32, tag="mb")
        bq_t = constp.tile([128, KC], F32, tag="bq")
        bk_t = constp.tile([128, KC], F32, tag="bk")
        bp_t = constp.tile([128, D], F32, tag="bp")
        wp_b = wpp.tile([128, KC, D], F16, tag="wpb")
        wp_t = [wp_b[:, k, :] for k in range(KC)]
        KT = [kvp.tile([128, S], F16, tag=f"KT{m}", name=f"KT{m}")
              for m in range(KC)]
        # V_aug: [128, pair, 192] = [V_even | ones(64) | V_odd]
        VA = [kvp.tile([128, NP, 192], F16, tag=f"VA{s}", name=f"VA{s}")
              for s in range(SC)]
        QT = [qtp.tile([128, R], F16, tag=f"QT{m}", name=f"QT{m}")
              for m in range(KC)]
        OT = [otp.tile([128, R], F16, tag=f"OT{m}", name=f"OT{m}")
              for m in range(KC)]
        # softmax denominators, head-aligned with OT rows: [rc, hp]
        sums_t = sumsp.tile([128, 2, NP, 512], F16, tag="sums")

        # ---- input DMAs, spread over the 3 DGE queues (~140GB/s each)
        # by compute deadline: ctx halves + wk column blocks feed the
        # k_proj stream from ~12us; xT halves + wq_m0 gate q_proj(0);
        # wv/wp/bp arrive under the attention stream.  V_aug's ones
        # blocks come from a DVE memset instead of a load.
        CB = KC * 128

        def wkj(j):
            return (wk_b[:, j, :, :], wk[:, j * CB:(j + 1) * CB])

        def wqj(j):
            return (wq_b[:, j, :, :], wq[:, j * CB:(j + 1) * CB])

        # exp0 is gated by ctx + xT + wk_m0 + wq_m0 (~4.75MB with the
        # early k blocks): those lead all three queues, deadline-order;
        # everything else (wq/wk rest, wv, wp, bp) streams in behind.
        nc.sync.dma_start(ctx_b[:], ctxT[:])
        nc.sync.dma_start(*wkj(0))
        nc.sync.dma_start(*wkj(1))
        nc.sync.dma_start(*wkj(6))
        nc.sync.dma_start(*wkj(7))
        nc.sync.dma_start(wv_b[:, 0, :, :], wv[:, 0:KC * 512])
        nc.gpsimd.dma_start(xT_b[:, 0:4, :], xT[:, 0:4 * R])
        nc.gpsimd.dma_start(*wqj(0))
        nc.gpsimd.dma_start(*wkj(2))
        nc.gpsimd.dma_start(*wkj(3))
        nc.gpsimd.dma_start(*wqj(1))
        nc.gpsimd.dma_start(*wqj(2))
        nc.gpsimd.dma_start(*wqj(3))
        nc.gpsimd.dma_start(wv_b[:, 1, :, :], wv[:, KC * 512:])
        nc.scalar.dma_start(bq_t[:], bq[:])
        nc.scalar.dma_start(bk_t[:], bk[:])
        nc.scalar.dma_start(mb_t[:], maskb[:])
        nc.scalar.dma_start(xT_b[:, 4:8, :], xT[:, 4 * R:])
        nc.scalar.dma_start(*wkj(4))
        nc.scalar.dma_start(*wkj(5))
        for j in range(4, KC):
            nc.scalar.dma_start(*wqj(j))
        nc.scalar.dma_start(bp_t[:], bp_r[:])
        nc.scalar.dma_start(wp_b[:], wp[:])
        for s in range(SC):
            nc.vector.memset(VA[s][:, :, 64:128], 2.0 ** -10)

        def q_proj(m, rc):
            ps = psp.tile([128, 512], F32, tag="ps")
            for k in range(KC):
                nc.tensor.matmul(
                    ps[:], wq_b[:, m, k, :],
                    xT_t[k][:, rc * 512:(rc + 1) * 512],
                    start=(k == 0), stop=(k == KC - 1))
            nc.vector.tensor_scalar_add(
                QT[m][:, rc * 512:(rc + 1) * 512], ps[:], bq_t[:, m:m + 1])

        def k_proj(m):
            ps = psp.tile([128, S], F32, tag="ps")
            for k in range(KC):
                nc.tensor.matmul(ps[:], wk_b[:, m, k, :],
                                 ctx_t[k][:],
                                 start=(k == 0), stop=(k == KC - 1))
            nc.vector.tensor_scalar_add(KT[m][:], ps[:], bk_t[:, m:m + 1])

        def v_proj(n, s):
            ps = psp.tile([128, 512], F32, tag="ps")
            for k in range(KC):
                nc.tensor.matmul(ps[:], ctx_t[k][:, s * 128:(s + 1) * 128],
                                 wv_b[:, n, k, :],
                                 start=(k == 0), stop=(k == KC - 1))
            # scatter 8 heads (4 pairs) into V_aug blocks
            vsrc = ps[:].rearrange("p (h c) -> p h c", c=64)
            nc.vector.tensor_copy(VA[s][:, 4 * n:4 * n + 4, 0:64],
                                  vsrc[:, 0::2, :])
            nc.vector.tensor_copy(VA[s][:, 4 * n:4 * n + 4, 128:192],
                                  vsrc[:, 1::2, :])

        def qk_slot(hp, ex, rc, s):
            """One (rc, s) score block: 2 concurrent row-group matmuls
            + 2 Exps.  Fill work is emitted between slots so the PE
            FIFO never head-of-line-blocks the ACT Exp stream."""
            pss = []
            for e in range(2):
                lo, hi = 64 * e, 64 * e + 64
                ps = psp.tile([128, 512], F32, tag="ps",
                              name=f"psqk{hp}_{s}_{rc}_{e}")
                nc.tensor.matmul(
                    ps[:],
                    KT[hp][lo:hi, s * 128:(s + 1) * 128],
                    QT[hp][lo:hi, rc * 512:(rc + 1) * 512],
                    start=True, stop=True)
                pss.append(ps)
            for e in range(2):
                nc.scalar.activation(ex[e][rc][s][:], pss[e][:],
                                     AF.Exp, bias=mb_t[:, s:s + 1])

        def attn_av(hp, ex, rc, e):
            rr = slice(rc * 512, rc * 512 + 512)
            # even head: V cols 0:128 -> O rows 0:64, sums 64:128
            # odd  head: V cols 64:192 -> sums 0:64, O rows 64:128
            voff = 64 * e
            olo, ohi = (0, 64) if e == 0 else (64, 128)
            slo, shi = (64, 128) if e == 0 else (0, 64)
            ps = psAV.tile([128, 512], F32, tag="psAV")
            for s in range(SC):
                nc.tensor.matmul(
                    ps[:], VA[s][:, hp, voff:voff + 128],
                    ex[e][rc][s][:],
                    start=(s == 0), stop=(s == SC - 1))
            nc.vector.tensor_copy(OT[hp][olo:ohi, rr],
                                  ps[olo:ohi, :])
            # cross-partition evict: head hp's sums land in the
            # opposite 64-half; write them aligned with its O
            nc.vector.tensor_copy(sums_t[olo:ohi, rc, hp, :],
                                  ps[slo:shi, :])

        # ============ head-streamed pipeline ============
        # q_proj halves (qc), v_proj chains (vc) and av pairs (ac) are
        # dribbled between the (rc, s) score slots from this ordered
        # work list, so neither the PE FIFO nor the Exp stream ever
        # stalls the other for more than ~1us.
        def qc(m, rc):
            return lambda ex: q_proj(m, rc)

        def kc(m):
            return lambda ex: k_proj(m)

        def vc(n, s):
            return lambda ex: v_proj(n, s)

        def ac(hp, rc, e):
            return lambda ex: attn_av(hp, ex[hp], rc, e)

        work = [
            qc(0, 1), kc(4), qc(1, 0), kc(5), qc(1, 1), kc(6),
            kc(7), qc(2, 0), vc(0, 0), qc(2, 1), vc(0, 1),
            vc(0, 2), qc(3, 0), vc(0, 3), qc(3, 1),
            ac(0, 0, 0), ac(0, 0, 1), qc(4, 0), ac(0, 1, 0), ac(0, 1, 1),
            qc(4, 1), ac(1, 0, 0), ac(1, 0, 1), qc(5, 0), ac(1, 1, 0),
            ac(1, 1, 1), qc(5, 1), ac(2, 0, 0), ac(2, 0, 1), qc(6, 0),
            ac(2, 1, 0), ac(2, 1, 1), qc(6, 1), vc(1, 0), ac(3, 0, 0),
            ac(3, 0, 1), qc(7, 0), vc(1, 1), ac(3, 1, 0), ac(3, 1, 1),
            qc(7, 1), vc(1, 2), vc(1, 3), ac(4, 0, 0), ac(4, 0, 1),
            ac(4, 1, 0), ac(4, 1, 1), ac(5, 0, 0), ac(5, 0, 1),
            ac(5, 1, 0), ac(5, 1, 1), ac(6, 0, 0), ac(6, 0, 1),
            ac(6, 1, 0), ac(6, 1, 1), ac(7, 0, 0), ac(7, 0, 1),
        ]
        for m in range(4):
            k_proj(m)
        q_proj(0, 0)
        exs = {}
        slot = 0
        popped = 0
        for hp in range(NP):
            exs[hp] = [[[expp.tile([128, 512], F16, tag="exp",
                                   name=f"ex{hp}_{e}_{rc}_{s}")
                         for s in range(SC)] for rc in range(2)]
                       for e in range(2)]
            for rc in range(2):
                for s in range(SC):
                    qk_slot(hp, exs[hp], rc, s)
                    slot += 1
                    want = (slot * len(work)) // (NP * 2 * SC)
                    while popped < want:
                        work[popped](exs)
                        popped += 1
        while popped < len(work):
            work[popped](exs)
            popped += 1
        # recip rc0 on ACT overlaps av(7, rc1) on the PE
        _act_recip(nc, sums_t[:, 0, :, :], sums_t[:, 0, :, :])
        attn_av(NP - 1, exs[NP - 1], 1, 0)
        attn_av(NP - 1, exs[NP - 1], 1, 1)
        for hp in range(NP):
            nc.vector.tensor_mul(OT[hp][:, 0:512], OT[hp][:, 0:512],
                                 sums_t[:, 0, hp, :])
        _act_recip(nc, sums_t[:, 1, :, :], sums_t[:, 1, :, :])
        for hp in range(NP):
            nc.vector.tensor_mul(OT[hp][:, 512:1024], OT[hp][:, 512:1024],
                                 sums_t[:, 1, hp, :])

        p_psAV.__exit__(None, None, None)
        p_ps.__exit__(None, None, None)
        p_exp.__exit__(None, None, None)

        # ================= output projection =================
        p_psD = tc.tile_pool(name="psD", bufs=5, space="PSUM")
        psD = p_psD.__enter__()
        p_y = tc.tile_pool(name="y", bufs=4)
        yp = p_y.__enter__()
        dma_engs = [nc.sync, nc.gpsimd, nc.scalar]
        for rp in range(KC):
            for n in range(2):
                ps = psD.tile([128, 512], F32, tag="psD")
                for k in range(KC):
                    nc.tensor.matmul(
                        ps[:], OT[k][:, rp * 128:(rp + 1) * 128],
                        wp_t[k][:, n * 512:(n + 1) * 512],
                        start=(k == 0), stop=(k == KC - 1))
                yt = yp.tile([128, 512], F32, tag="y")
                nc.vector.tensor_add(yt[:], ps[:], bp_t[:, n * 512:(n + 1) * 512])
                i = rp * 2 + n
                dma_engs[i % 3].dma_start(
                    y[rp * 128:rp * 128 + 64, n * 512:(n + 1) * 512],
                    yt[0:64, :])
                dma_engs[(i + 1) % 3].dma_start(
                    y[rp * 128 + 64:(rp + 1) * 128, n * 512:(n + 1) * 512],
                    yt[64:128, :])
        p_y.__exit__(None, None, None)
        p_psD.__exit__(None, None, None)
        p_ld.__exit__(None, None, None)
        p_sums.__exit__(None, None, None)
        p_wp.__exit__(None, None, None)
        p_ot.__exit__(None, None, None)
        p_qt.__exit__(None, None, None)
        p_kv.__exit__(None, None, None)
        p_const.__exit__(None, None, None)

    nc.compile()
    return nc


def _get_nc():
    if "nc" not in _CACHE:
        _CACHE["nc"] = _build()
    return _CACHE["nc"]


def kernel(x, context, context_mask, Wq, bq, Wkv, bkv, Wp, bp):
    global last_results
    x = np.asarray(x, dtype=np.float32)
    context = np.asarray(context, dtype=np.float32)
    context_mask = np.asarray(context_mask)
    Wq = np.asarray(Wq, dtype=np.float32)
    bq = np.asarray(bq, dtype=np.float32)
    Wkv = np.asarray(Wkv, dtype=np.float32)
    bkv = np.asarray(bkv, dtype=np.float32)
    Wp = np.asarray(Wp, dtype=np.float32)
    bp = np.asarray(bp, dtype=np.float32)

    sc = 1.0 / np.sqrt(HD)

    def pack(a):
        # [D, X] row-chunked -> [128, KC, X] partition-major: big
        # contiguous DMA lines
        Dl, X = a.shape
        return np.ascontiguousarray(
            a.reshape(Dl // 128, 128, X).transpose(1, 0, 2)
            .reshape(128, (Dl // 128) * X))

    def pack_cols(a, w):
        # [D, D2] -> [128, j, k, w] so column block j (all row chunks
        # k) is one contiguous 2*KC*w*128-byte DMA
        Dl, D2 = a.shape
        return np.ascontiguousarray(
            a.reshape(Dl // 128, 128, D2 // w, w).transpose(1, 2, 0, 3)
            .reshape(128, -1))

    # kv reshape in the reference is [S, 2, H, Hd]: k cols = Wkv[:, :D]
    wq_h = pack_cols((Wq * sc).astype(np.float16), 128)
    bq_h = np.ascontiguousarray((bq * sc).reshape(KC, 128).T)
    wk_h = pack_cols(Wkv[:, :D].astype(np.float16), 128)
    bk_h = np.ascontiguousarray(bkv[:D].reshape(KC, 128).T)
    wv_h = pack_cols((Wkv[:, D:] * 2.0**-10).astype(np.float16), 512)
    bv = bkv[D:]
    wp_h = pack(Wp.astype(np.float16))
    bp_eff = bp + bv @ Wp          # softmax rows sum to 1
    bp_r = np.ascontiguousarray(
        np.broadcast_to(bp_eff.astype(np.float32), (128, D)))

    in_maps = []
    for c in range(NCORE):
        b = c // 2
        r0 = (c % 2) * R
        in_maps.append({
            "xT": pack(x[b, r0:r0 + R, :].T.astype(np.float16)),
            "ctxT": pack(context[b].T.astype(np.float16)),
            "maskb": np.ascontiguousarray(
                np.where(context_mask[b], 0.0, NEG).astype(np.float32)
                .reshape(SC, 128).T),
            "wq": wq_h, "bq": bq_h,
            "wk": wk_h, "bk": bk_h,
            "wv": wv_h,
            "wp": wp_h, "bp_r": bp_r,
        })

    nc = _get_nc()
    res = run_bass_kernel_spmd(nc, in_maps, list(range(NCORE)),
                               trace=bool(os.environ.get("BASS_TRACE")))
    last_results = res

    out = np.empty((B, T, D), dtype=np.float32)
    for c in range(NCORE):
        b = c // 2
        r0 = (c % 2) * R
        out[b, r0:r0 + R, :] = res.results[c]["y"]
    return out
